# revision 3
# baseline (speedup 1.0000x reference)
"""Bass/Trainium2 kernel for nn_EquivariantReynoldsWrap.

The reference module is linear in x: for every pixel,
    out = (1/G) * sum_g BlockDiag(A_g) @ Wf @ BlockDiag(Ainv_g) @ x_pixel
so the whole pipeline collapses into one 64x64 channel-mixing matrix M,
computed on host (cheap). The device work is a single 1x1-conv matmul
out[b] = M @ x[b] with x[b] viewed as (64, H*W).

Sharding: data-parallel over B across the 8 cores (1 batch each).
Per core we stack the two halves of the pixel axis on the partition axis
(128 partitions) and use a block-diagonal 128x128 stationary weight
blockdiag(M^T, M^T), so each 512-column matmul covers 1024 pixels.
"""

import numpy as np

import concourse.bacc as bacc
import concourse.bass as bass
import concourse.tile as tile
from concourse import mybir
from concourse.bass_utils import run_bass_kernel_spmd

B, C, H, W_SP = 8, 64, 64, 64
COUT = 64
HW = H * W_SP          # 4096 pixels per batch
HALF = HW // 2         # 2048 -> stacked column count per core
N_CORES = 8

TRACE = False          # test.py flips this to profile
_cached_nc = None


def _build_nc():
    global _cached_nc
    if _cached_nc is not None:
        return _cached_nc

    nc = bacc.Bacc(
        "TRN2",
        target_bir_lowering=False,
        debug=False,
        enable_asserts=False,
        num_devices=N_CORES,
    )
    xd = nc.dram_tensor("x", [C, HW], mybir.dt.float32, kind="ExternalInput").ap()
    wd = nc.dram_tensor("w", [128, 128], mybir.dt.float32, kind="ExternalInput").ap()
    yd = nc.dram_tensor("y", [COUT, HW], mybir.dt.float32, kind="ExternalOutput").ap()

    CH = 1024  # columns per pipeline chunk

    with tile.TileContext(nc) as tc:
        with (
            tc.tile_pool(name="io", bufs=2) as io,
            tc.tile_pool(name="wp", bufs=1) as wp,
            tc.tile_pool(name="pp", bufs=2, space="PSUM") as pp,
        ):
            wt = wp.tile([128, 128], mybir.dt.float32)
            nc.sync.dma_start(wt[:], wd[:])

            # [2, 64, t] with s-major outer dims: DMA pairs partition p<64
            # with (s=0, c=p) = pixels [0, HALF) and p>=64 with (s=1, c=p-64)
            # = pixels [HALF, HW)
            xr = xd.rearrange("c (s t) -> c s t", s=2).transpose([1, 0, 2])
            yr = yd.rearrange("c (s t) -> c s t", s=2).transpose([1, 0, 2])

            for i in range(HALF // CH):
                xt = io.tile([128, CH], mybir.dt.float32, tag="xt")
                nc.sync.dma_start(xt[:], xr[:, :, bass.ts(i, CH)])

                ps = pp.tile([128, CH], mybir.dt.float32)
                for j in range(CH // 512):
                    nc.tensor.matmul(
                        ps[:, bass.ts(j, 512)],
                        wt[:],
                        xt[:, bass.ts(j, 512)],
                        start=True,
                        stop=True,
                    )

                ot = io.tile([128, CH], mybir.dt.float32, tag="ot")
                half = CH // 2
                nc.vector.tensor_copy(ot[:, :half], ps[:, :half])
                nc.scalar.copy(ot[:, half:], ps[:, half:])

                nc.sync.dma_start(yr[:, :, bass.ts(i, CH)], ot[:])

    nc.compile()
    _cached_nc = nc
    return nc


def _fuse_weights(group_tensor, group_tensor_inv, Wf):
    A = np.asarray(group_tensor, np.float64)
    Ai = np.asarray(group_tensor_inv, np.float64)
    Wf64 = np.asarray(Wf, np.float64)
    G, CG, _ = A.shape
    n = C // CG
    eye = np.eye(n)
    M = np.zeros((COUT, C))
    for g in range(G):
        M += np.kron(eye, A[g]) @ Wf64 @ np.kron(eye, Ai[g])
    M /= G
    MT = np.ascontiguousarray(M.T).astype(np.float32)
    W2T = np.zeros((128, 128), np.float32)
    W2T[:64, :64] = MT
    W2T[64:, 64:] = MT
    return W2T


def kernel(x, group_tensor, group_tensor_inv, Wf):
    nc = _build_nc()
    W2T = _fuse_weights(group_tensor, group_tensor_inv, Wf)
    x = np.ascontiguousarray(np.asarray(x, np.float32))

    in_maps = [
        {"x": x[b].reshape(C, HW), "w": W2T} for b in range(B)
    ]
    res = run_bass_kernel_spmd(
        nc, in_maps, core_ids=list(range(N_CORES)), trace=TRACE
    )
    if TRACE:
        kernel.last_results = res
    y = np.stack(
        [res.results[b]["y"].reshape(COUT, H, W_SP) for b in range(B)]
    )
    return y


# revision 5
# speedup vs baseline: 2.0723x; 2.0723x over previous
"""Bass/Trainium2 kernel for nn_EquivariantReynoldsWrap.

The reference module is linear in x: for every pixel,
    out = (1/G) * sum_g BlockDiag(A_g) @ Wf @ BlockDiag(Ainv_g) @ x_pixel
so the whole pipeline collapses into one 64x64 channel-mixing matrix M,
computed on host (cheap). The device work is a single 1x1-conv matmul
out[b] = M @ x[b] with x[b] viewed as (64, H*W).

Sharding: data-parallel over B across the 8 cores (1 batch each).
Per core we stack the two halves of the pixel axis on the partition axis
(128 partitions) and use a block-diagonal 128x128 stationary weight
blockdiag(M^T, M^T), so each 512-column matmul covers 1024 pixels.
"""

import numpy as np

import concourse.bacc as bacc
import concourse.bass as bass
import concourse.tile as tile
from concourse import mybir
from concourse.bass_utils import run_bass_kernel_spmd

B, C, H, W_SP = 8, 64, 64, 64
COUT = 64
HW = H * W_SP          # 4096 pixels per batch
HALF = HW // 2         # 2048 -> stacked column count per core
N_CORES = 8

TRACE = False          # test.py flips this to profile
_cached_nc = None


def _build_nc():
    global _cached_nc
    if _cached_nc is not None:
        return _cached_nc

    nc = bacc.Bacc(
        "TRN2",
        target_bir_lowering=False,
        debug=False,
        enable_asserts=False,
        num_devices=N_CORES,
    )
    xd = nc.dram_tensor("x", [C, HW], mybir.dt.float32, kind="ExternalInput").ap()
    wd = nc.dram_tensor("w", [128, 128], mybir.dt.float32, kind="ExternalInput").ap()
    yd = nc.dram_tensor("y", [COUT, HW], mybir.dt.float32, kind="ExternalOutput").ap()

    CH = 1024  # columns per pipeline chunk

    with tile.TileContext(nc) as tc:
        with (
            tc.tile_pool(name="io", bufs=2) as io,
            tc.tile_pool(name="wp", bufs=1) as wp,
            tc.tile_pool(name="pp", bufs=2, space="PSUM") as pp,
        ):
            wt = wp.tile([128, 128], mybir.dt.float32)
            nc.sync.dma_start(wt[:], wd[:])

            # [64, 2, t] c-major outer dims: DMA pairs partition p with
            # (c=p//2, s=p%2); the outer dim of 64 spreads the transfer
            # across all 16 SDMA engines (outer dim 2 used only 2).
            xr = xd.rearrange("c (s t) -> c s t", s=2)
            yr = yd.rearrange("c (s t) -> c s t", s=2)

            for i in range(HALF // CH):
                xt = io.tile([128, CH], mybir.dt.float32, tag="xt")
                nc.sync.dma_start(xt[:], xr[:, :, bass.ts(i, CH)])

                ps = pp.tile([128, CH], mybir.dt.float32)
                for j in range(CH // 512):
                    nc.tensor.matmul(
                        ps[:, bass.ts(j, 512)],
                        wt[:],
                        xt[:, bass.ts(j, 512)],
                        start=True,
                        stop=True,
                    )

                ot = io.tile([128, CH], mybir.dt.float32, tag="ot")
                half = CH // 2
                nc.vector.tensor_copy(ot[:, :half], ps[:, :half])
                nc.scalar.copy(ot[:, half:], ps[:, half:])

                nc.sync.dma_start(yr[:, :, bass.ts(i, CH)], ot[:])

    nc.compile()
    _cached_nc = nc
    return nc


def _fuse_weights(group_tensor, group_tensor_inv, Wf):
    A = np.asarray(group_tensor, np.float64)
    Ai = np.asarray(group_tensor_inv, np.float64)
    Wf64 = np.asarray(Wf, np.float64)
    G, CG, _ = A.shape
    n = C // CG
    eye = np.eye(n)
    M = np.zeros((COUT, C))
    for g in range(G):
        M += np.kron(eye, A[g]) @ Wf64 @ np.kron(eye, Ai[g])
    M /= G
    MT = np.ascontiguousarray(M.T).astype(np.float32)
    # interleaved packing: x-tile partition p holds channel p//2 of pixel
    # half p%2; out partition q holds channel q//2 of half q%2.
    W2T = np.zeros((128, 128), np.float32)
    W2T[0::2, 0::2] = MT
    W2T[1::2, 1::2] = MT
    return W2T


def kernel(x, group_tensor, group_tensor_inv, Wf):
    nc = _build_nc()
    W2T = _fuse_weights(group_tensor, group_tensor_inv, Wf)
    x = np.ascontiguousarray(np.asarray(x, np.float32))

    in_maps = [
        {"x": x[b].reshape(C, HW), "w": W2T} for b in range(B)
    ]
    res = run_bass_kernel_spmd(
        nc, in_maps, core_ids=list(range(N_CORES)), trace=TRACE
    )
    if TRACE:
        kernel.last_results = res
    y = np.stack(
        [res.results[b]["y"].reshape(COUT, H, W_SP) for b in range(B)]
    )
    return y
